# revision 13
# baseline (speedup 1.0000x reference)
"""Trainium2 Bass kernel v4 for the encoder block.

Data-parallel over batch (1 element/core, no collectives).

vs v3:
- DMAs coalesced (one trigger per tensor; the sync sequencer costs
  ~650ns per dma_start) and ordered so x8T/wv8 land first; bulky
  ff-weights stream during attention.
- v_aug pad memsets moved to gpsimd and shrunk to the pad columns.
- proj/LN1 (phase D) interleaved with ff1 halves to keep the tensor
  engine fed across the transition; LN applies run on gpsimd and
  x1T8 psum evacuations on ACT to unload DVE.
- ff2 contraction split: first 2*FP8_PAIRS ct-chunks run fp8
  DoubleRow (gelu emits unscaled fp8 g; w2 pre-scaled by 512), the
  remaining chunks stay bf16 (also 512x so the psum scale matches).
  FP8_PAIRS=9 keeps rel-err ~1.7e-2 (sim) while cutting ff2 ~37%.

Scale bookkeeping: x8T = fp8(x)^T; wq/k/r/v/proj/ff1 = fp8(32*w);
q,k evac 1/32 (bf16 true), r keeps 32x; v_aug8 = fp8(2048*v/sqrt(E)),
ones col 32; og8 = psum_d*r_sb/(32*psum_one) = 64*(r*og)_true;
proj dequant 1/2048 fused into residual; ff2 psum = 512*ff ->
dequant 1/512 fused into residual.
"""

import sys

if "/opt/trn_rl_repo" not in sys.path:
    sys.path.insert(0, "/opt/trn_rl_repo")

from contextlib import ExitStack

import numpy as np
import ml_dtypes

import concourse.bass as bass
import concourse.mybir as mybir
import concourse.tile as tile
from concourse import bacc
from concourse.bass_utils import run_bass_kernel_spmd
from concourse.masks import make_identity

F32 = mybir.dt.float32
BF16 = mybir.dt.bfloat16
FP8 = mybir.dt.float8e4
AF = mybir.ActivationFunctionType
ALU = mybir.AluOpType
DR = mybir.MatmulPerfMode.DoubleRow

N_CORES = 8
B, N, E = 8, 1024, 768
H, D = 8, 96
C = 4 * E
NQT = N // 128
NEC = E // 128
NCT = C // 128
LN_EPS = 1e-5

WS = 32.0
INV_WS = 1.0 / WS
C_SHIFT = 12.0       # exp(e - C_SHIFT): emax ~17.3 -> exp' <= ~210 < 448
ONES_VAL = 32.0
VA_SCALE = 2048.0
SUM_EPS = 1e-6
PROJ_DEQ = 1.0 / (64.0 * 32.0)
FF2_SCALE = 512.0    # w2 pre-scale (both fp8 and bf16 parts)
FF2_DEQ = 1.0 / FF2_SCALE
FP8_PAIRS = 9        # ct-pairs of ff2 contraction done in fp8 DR (0..12)


def _bcast_dma(nc, out_ap, row_ap):
    src = bass.AP(
        tensor=row_ap.tensor,
        offset=row_ap.offset,
        ap=[[0, out_ap.shape[0]], list(row_ap.ap[-1])],
    )
    nc.gpsimd.dma_start(out=out_ap, in_=src)


def _ln_stats_norm(nc, pool, t1, out, eps_t, g_bc, b_bc, identity_ln, tag):
    """LN over free dim 768 of t1 -> out. Stats on DVE, apply on gpsimd."""
    scr = pool.tile([128, 32], F32, tag=f"lns_{tag}", name=f"lns_{tag}")
    st = scr[:, 0:18].rearrange("p (a b) -> p a b", a=3)
    mv = scr[:, 24:26]
    rstd = scr[:, 26:27]
    t2 = out if identity_ln else pool.tile([128, E], F32, tag=f"lnt2_{tag}", name=f"lnt2_{tag}")
    for sg in range(3):
        nc.vector.bn_stats(st[:, sg, :], t1[:, sg * 256 : (sg + 1) * 256])
    nc.vector.bn_aggr(mv, st)
    nc.scalar.activation(out=rstd, in_=mv[:, 1:2], func=AF.Sqrt, bias=eps_t[:], scale=1.0)
    nc.vector.reciprocal(rstd, rstd)
    nc.vector.tensor_scalar(
        out=t2[:] if t2 is not out else t2, in0=t1, scalar1=mv[:, 0:1], scalar2=rstd,
        op0=ALU.subtract, op1=ALU.mult,
    )
    if not identity_ln:
        nc.vector.tensor_tensor(out=t2[:], in0=t2[:], in1=g_bc, op=ALU.mult)
        nc.vector.tensor_tensor(out=out, in0=t2[:], in1=b_bc, op=ALU.add)


def _build(identity_ln=False, zero_bias=False):
    nc = bacc.Bacc(num_devices=N_CORES)
    NF8 = 2 * FP8_PAIRS          # ct chunks done in fp8
    NB16 = NCT - NF8             # ct chunks done in bf16

    x8t_d = nc.declare_dram_parameter("x8t", [128, NEC, N], FP8, isOutput=False)
    xb16_d = nc.declare_dram_parameter("xb16", [N, E], BF16, isOutput=False)
    wqkr_d = nc.declare_dram_parameter("wqkr", [128, H, 3, 3, 2, D], FP8, isOutput=False)
    wv_d = nc.declare_dram_parameter("wv", [128, 3, 2, E], FP8, isOutput=False)
    wproj_d = nc.declare_dram_parameter("wproj", [D, 4, 2, E], FP8, isOutput=False)
    bqkr_d = nc.declare_dram_parameter("bqkr", [D, 3, H], F32, isOutput=False)
    bv_d = nc.declare_dram_parameter("bv", [1, E], F32, isOutput=False)
    bproj_d = nc.declare_dram_parameter("bproj", [1, E], F32, isOutput=False)
    ln1g_d = nc.declare_dram_parameter("ln1g", [1, E], F32, isOutput=False)
    ln1b_d = nc.declare_dram_parameter("ln1b", [1, E], F32, isOutput=False)
    wff1_d = nc.declare_dram_parameter("wff1", [128, NCT, 3, 2, 128], FP8, isOutput=False)
    bff1_d = nc.declare_dram_parameter("bff1", [128, NCT], F32, isOutput=False)
    wff2_d = nc.declare_dram_parameter("wff2", [C, E], BF16, isOutput=False)
    wff28_d = nc.declare_dram_parameter("wff28", [128, NCT // 2, 2, E], FP8, isOutput=False)
    bff2_d = nc.declare_dram_parameter("bff2", [1, E], F32, isOutput=False)
    ln2g_d = nc.declare_dram_parameter("ln2g", [1, E], F32, isOutput=False)
    ln2b_d = nc.declare_dram_parameter("ln2b", [1, E], F32, isOutput=False)
    y_d = nc.declare_dram_parameter("y", [N, E], F32, isOutput=True)

    with tile.TileContext(nc) as tc, ExitStack() as ctx:
        persist = ctx.enter_context(tc.tile_pool(name="persist", bufs=1))
        vaug_pool = ctx.enter_context(tc.tile_pool(name="vaug", bufs=1))
        og_pool = ctx.enter_context(tc.tile_pool(name="og", bufs=1))
        x1_pool = ctx.enter_context(tc.tile_pool(name="x1", bufs=1))
        x1t_pool = ctx.enter_context(tc.tile_pool(name="x1t", bufs=1))

        # critical-path DMAs first (one trigger each)
        x8T = persist.tile([128, NEC, N], FP8, name="x8T")
        nc.sync.dma_start(out=x8T[:], in_=x8t_d[:])
        wv8 = persist.tile([128, 3, 2, E], FP8, name="wv8")
        nc.sync.dma_start(out=wv8[:], in_=wv_d[:])
        bqkr_t = persist.tile([D, 3, H], F32)
        nc.sync.dma_start(out=bqkr_t[:], in_=bqkr_d[:])
        wp8 = persist.tile([D, 4, 2, E], FP8, name="wp8")
        nc.sync.dma_start(out=wp8[:], in_=wproj_d[:])

        identb = persist.tile([128, 128], BF16)
        make_identity(nc, identb[:])
        eps_t = persist.tile([128, 1], F32)
        nc.vector.memset(eps_t[:], LN_EPS)
        shift_t = persist.tile([128, 1], F32)
        nc.vector.memset(shift_t[:], -C_SHIFT)

        # PE warm-up while DMAs land
        warm_t = persist.tile([128, 128], BF16)
        nc.vector.memset(warm_t[:], 0.0)
        with tc.tile_pool(name="warm_ps", bufs=2, space="PSUM") as warm_ps:
            for _ in range(8):
                wp_ = warm_ps.tile([128, 128], F32, tag="wp_", name="wp_")
                nc.tensor.matmul(wp_[:], warm_t[:], warm_t[:], start=True, stop=True)
                nc.tensor.matmul(wp_[:], warm_t[:], warm_t[:], start=True, stop=True)

        # v_aug pad + ones cols (gpsimd, off the DVE critical path)
        v_aug = vaug_pool.tile([128, H, NQT, 128], FP8, name="v_aug")
        nc.gpsimd.memset(v_aug[:, :, :, D:], 0.0)
        nc.gpsimd.memset(v_aug[:, :, :, D : D + 1], ONES_VAL)

        # bulky ff-phase loads: issue now, they stream during attention
        xb16 = persist.tile([128, NQT, E], BF16, name="xb16")
        src = bass.AP(tensor=xb16_d.tensor if hasattr(xb16_d, 'tensor') else xb16_d, offset=0,
                      ap=[[E, 128], [128 * E, NQT], [1, E]])
        nc.sync.dma_start(out=xb16[:], in_=src)
        wff1sb = persist.tile([128, NCT, 3, 2, 128], FP8, name="wff1sb")
        nc.sync.dma_start(out=wff1sb[:], in_=wff1_d[:])
        bff1_t = persist.tile([128, NCT], F32, name="bff1_t")
        nc.sync.dma_start(out=bff1_t[:], in_=bff1_d[:])
        w28sb = None
        if FP8_PAIRS > 0:
            w28sb = persist.tile([128, FP8_PAIRS, 2, E], FP8, name="w28sb")
            nc.sync.dma_start(out=w28sb[:], in_=wff28_d[:, 0:FP8_PAIRS])
        NF8_ = NF8
        w2sb = None
        if NB16 > 0:
            w2sb = persist.tile([128, NB16, E], BF16, name="w2sb")
            src2 = bass.AP(tensor=wff2_d.tensor if hasattr(wff2_d, 'tensor') else wff2_d,
                           offset=NF8_ * 128 * E,
                           ap=[[E, 128], [128 * E, NB16], [1, E]])
            nc.sync.dma_start(out=w2sb[:], in_=src2)

        # ---- phase B: v = x @ wv (fp8 DR) -> v_aug8 ----
        VSC = VA_SCALE / (WS * float(np.sqrt(E)))
        with (
            tc.tile_pool(name="bcv", bufs=1) as bcv_pool,
            tc.tile_pool(name="v_ps", bufs=2, space="PSUM") as v_ps,
        ):
            bv_bc = None
            if not zero_bias:
                bv_bc = bcv_pool.tile([128, E], F32, tag="bv", name="bv_bc")
                _bcast_dma(nc, bv_bc[:], bv_d[0:1, :])
            for qt in range(NQT):
                vps = v_ps.tile([128, 1024], F32, tag="vp", name="vp")
                for o, w in ((0, 512), (512, 256)):
                    for kc in range(3):
                        nc.tensor.matmul(
                            vps[:, o : o + w],
                            x8T[:, 2 * kc : 2 * kc + 2, qt * 128 : (qt + 1) * 128],
                            wv8[:, kc, :, o : o + w],
                            start=(kc == 0), stop=(kc == 2), perf_mode=DR,
                        )
                dst = v_aug[:, :, qt, 0:D]
                src_v = vps[:, 0:E].rearrange("p (h d) -> p h d", h=H)
                if zero_bias:
                    nc.scalar.activation(
                        out=dst, in_=src_v, func=AF.Copy, bias=0.0, scale=VSC,
                    )
                else:
                    nc.vector.scalar_tensor_tensor(
                        out=dst, in0=src_v, scalar=VSC,
                        in1=bv_bc[:, 0:E].rearrange("p (h d) -> p h d", h=H),
                        op0=ALU.mult, op1=ALU.add,
                    )

        # ---- phase C: attention, software-pipelined over heads ----
        og8 = og_pool.tile([D, H, N], FP8, name="og8")
        with (
            tc.tile_pool(name="wqkr", bufs=2) as wqkr_pool,
            tc.tile_pool(name="qkr", bufs=2) as qkr_pool,
            tc.tile_pool(name="expE", bufs=2) as exp_pool,
            tc.tile_pool(name="att_tmp", bufs=2) as tmp_pool,
            tc.tile_pool(name="qkr_ps", bufs=2, space="PSUM") as qkr_ps,
            tc.tile_pool(name="eng_ps", bufs=2, space="PSUM") as eng_ps,
            tc.tile_pool(name="att_ps", bufs=1, space="PSUM") as att_ps,
        ):
            def qkr_stage(h):
                w_sb = wqkr_pool.tile([128, 3, 3, 2, D], FP8, tag="w_qkr", name="w_qkr")
                nc.sync.dma_start(out=w_sb[:], in_=wqkr_d[:, h])
                qkrT = {}
                for si, name in enumerate(("q", "k", "r")):
                    dst = qkr_pool.tile([D, N], BF16, tag=f"{name}T", name=f"{name}T")
                    qkrT[name] = dst
                    for half in range(2):
                        ps = qkr_ps.tile([D, 512], F32, tag="qkrp", name="qkrp")
                        for kc in range(3):
                            nc.tensor.matmul(
                                ps[:],
                                w_sb[:, si, kc],
                                x8T[:, 2 * kc : 2 * kc + 2,
                                    half * 512 : (half + 1) * 512],
                                start=(kc == 0), stop=(kc == 2), perf_mode=DR,
                            )
                        out_sl = dst[:, half * 512 : (half + 1) * 512]
                        if name == "r":
                            if zero_bias:
                                nc.vector.tensor_copy(out_sl, ps[:])
                            else:
                                nc.vector.tensor_scalar(
                                    out=out_sl, in0=ps[:],
                                    scalar1=bqkr_t[:, si, h : h + 1], scalar2=None,
                                    op0=ALU.add,
                                )
                        else:
                            if zero_bias:
                                nc.vector.tensor_scalar(
                                    out=out_sl, in0=ps[:],
                                    scalar1=INV_WS, scalar2=None, op0=ALU.mult,
                                )
                            else:
                                nc.vector.tensor_scalar(
                                    out=out_sl, in0=ps[:],
                                    scalar1=INV_WS, scalar2=bqkr_t[:, si, h : h + 1],
                                    op0=ALU.mult, op1=ALU.add,
                                )
                return qkrT

            def energy_stage(h, qkrT):
                expE = exp_pool.tile([128, NQT, N], FP8, tag="expE", name="expE")
                for kt in range(NQT):
                    ep = eng_ps.tile([128, N], F32, tag="ep", name="ep")
                    for qh in range(2):
                        nc.tensor.matmul(
                            ep[:, qh * 512 : (qh + 1) * 512],
                            qkrT["k"][:, kt * 128 : (kt + 1) * 128],
                            qkrT["q"][:, qh * 512 : (qh + 1) * 512],
                            start=True, stop=True, skip_group_check=True,
                        )
                    nc.scalar.activation(
                        out=expE[:, kt, :], in_=ep[:], func=AF.Exp,
                        bias=shift_t[:], scale=1.0,
                    )
                return expE

            def attv_stage(h, expE):
                op_ = att_ps.tile([128, N], F32, tag="op", name="op")
                for j in range(NQT // 2):
                    for qh in range(2):
                        nc.tensor.matmul(
                            op_[:, qh * 512 : (qh + 1) * 512],
                            v_aug[:, h, 2 * j : 2 * j + 2, :],
                            expE[:, 2 * j : 2 * j + 2, qh * 512 : (qh + 1) * 512],
                            start=(j == 0), stop=(j == NQT // 2 - 1),
                            perf_mode=DR, skip_group_check=True,
                        )
                return op_

            def tail_stage(h, op_, qkrT):
                su = tmp_pool.tile([1, N], F32, tag="su", name="su")
                nc.vector.tensor_scalar(
                    out=su[:], in0=op_[D : D + 1, :],
                    scalar1=32.0, scalar2=SUM_EPS, op0=ALU.mult, op1=ALU.add,
                )
                gated = tmp_pool.tile([D, N], F32, tag="gated", name="gated")
                nc.vector.tensor_tensor(
                    out=gated[:], in0=op_[0:D, :], in1=qkrT["r"][:], op=ALU.mult,
                )
                rcp = tmp_pool.tile([1, N], F32, tag="rcp", name="rcp")
                nc.vector.reciprocal_approx_fast(rcp[:], su[:])
                bcr = tmp_pool.tile([D, N], F32, tag="bcr", name="bcr")
                nc.gpsimd.partition_broadcast(bcr[:], rcp[:])
                nc.vector.tensor_tensor(
                    out=og8[:, h, :], in0=gated[:], in1=bcr[:], op=ALU.mult,
                )

            prev = None
            for h in range(H):
                qkrT = qkr_stage(h)
                if prev is not None:
                    ph, pexpE, pqkrT = prev
                    op_ = attv_stage(ph, pexpE)
                    tail_stage(ph, op_, pqkrT)
                expE = energy_stage(h, qkrT)
                prev = (h, expE, qkrT)
            ph, pexpE, pqkrT = prev
            op_ = attv_stage(ph, pexpE)
            tail_stage(ph, op_, pqkrT)

        # ---- phases D+E interleaved: proj+LN1 / ff1 / ff2+LN2 ----
        x1b = x1_pool.tile([128, NQT, E], BF16, name="x1b")
        x1T8 = x1t_pool.tile([128, NEC, N], FP8, name="x1T8")
        with (
            tc.tile_pool(name="bcmid", bufs=1) as bcm_pool,
            tc.tile_pool(name="ln_tmp", bufs=2) as ln_pool,
            tc.tile_pool(name="gs8", bufs=2) as gs8_pool,
            tc.tile_pool(name="gsb", bufs=2) as gsb_pool,
            tc.tile_pool(name="ln2_tmp", bufs=2) as ln2_pool,
            tc.tile_pool(name="out", bufs=2) as out_pool,
            tc.tile_pool(name="h1_ps", bufs=3, space="PSUM") as h1_ps,
        ):
            bcm = bce = None
            if not (identity_ln and zero_bias):
                bcm = bcm_pool.tile([128, 3, E], F32, name="bcm")
                for i, d in enumerate((bproj_d, ln1g_d, ln1b_d)):
                    _bcast_dma(nc, bcm[:, i, :], d[0:1, :])
                bce = bcm_pool.tile([128, 3, E], F32, name="bce")
                for i, d in enumerate((bff2_d, ln2g_d, ln2b_d)):
                    _bcast_dma(nc, bce[:, i, :], d[0:1, :])

            def proj_ln(qt, y1_ps, tp1_ps):
                yp = y1_ps.tile([128, 1024], F32, tag="yp", name="yp")
                for o, w in ((0, 512), (512, 256)):
                    for hp in range(4):
                        nc.tensor.matmul(
                            yp[:, o : o + w],
                            og8[:, 2 * hp : 2 * hp + 2, qt * 128 : (qt + 1) * 128],
                            wp8[:, hp, :, o : o + w],
                            start=(hp == 0), stop=(hp == 3), perf_mode=DR,
                        )
                t1 = ln_pool.tile([128, E], F32, tag="t1", name="t1")
                xr = xb16[:, qt, :]
                if not zero_bias:
                    xrf = ln_pool.tile([128, E], F32, tag="xrf", name="xrf")
                    nc.vector.tensor_tensor(out=xrf[:], in0=xr, in1=bcm[:, 0, :], op=ALU.add)
                    xr = xrf[:]
                nc.vector.scalar_tensor_tensor(
                    out=t1[:], in0=yp[:, 0:E], scalar=PROJ_DEQ, in1=xr,
                    op0=ALU.mult, op1=ALU.add,
                )
                _ln_stats_norm(nc, ln_pool, t1[:], x1b[:, qt, :], eps_t,
                               bcm[:, 1, :] if bcm is not None else None,
                               bcm[:, 2, :] if bcm is not None else None,
                               identity_ln, "ln1")
                pt1 = tp1_ps.tile([128, NEC, 128], BF16, tag="pt1", name="pt1")
                for ec in range(NEC):
                    nc.tensor.transpose(pt1[:, ec, :], x1b[:, qt, ec * 128 : (ec + 1) * 128], identb[:])
                if qt < 4:
                    nc.scalar.activation(
                        out=x1T8[:, :, qt * 128 : (qt + 1) * 128], in_=pt1[:], func=AF.Copy,
                    )
                else:
                    nc.vector.tensor_copy(x1T8[:, :, qt * 128 : (qt + 1) * 128], pt1[:])

            def ff1_half(half):
                g8 = gs8_pool.tile([128, NF8, 512], FP8, tag="g8", name="g8") if NF8 else None
                gb = gsb_pool.tile([128, NB16, 512], BF16, tag="gb", name="gb") if NB16 else None
                for ct in range(NCT):
                    hp_ = h1_ps.tile([128, 512], F32, tag="h1", name="h1")
                    for kc in range(3):
                        nc.tensor.matmul(
                            hp_[:],
                            wff1sb[:, ct, kc],
                            x1T8[:, 2 * kc : 2 * kc + 2,
                                 half * 512 : (half + 1) * 512],
                            start=(kc == 0), stop=(kc == 2), perf_mode=DR,
                        )
                    if ct < NF8:
                        gdst = g8[:, ct, :]
                    else:
                        gdst = gb[:, ct - NF8, :]
                    nc.scalar.activation(
                        out=gdst, in_=hp_[:], func=AF.Gelu,
                        bias=bff1_t[:, ct : ct + 1], scale=INV_WS,
                    )
                return g8, gb

            gparts = [None, None]
            with (
                tc.tile_pool(name="y1_ps", bufs=2, space="PSUM") as y1_ps,
                tc.tile_pool(name="tp1_ps", bufs=1, space="PSUM") as tp1_ps,
            ):
                for qt in range(4):
                    proj_ln(qt, y1_ps, tp1_ps)
                gparts[0] = ff1_half(0)
                for qt in range(4, NQT):
                    proj_ln(qt, y1_ps, tp1_ps)
            gparts[1] = ff1_half(1)

            with tc.tile_pool(name="y2_ps", bufs=2, space="PSUM") as y2_ps:
                for half in range(2):
                    g8, gb = gparts[half]
                    for iq in range(4):
                        qt = half * 4 + iq
                        y2p = y2_ps.tile([128, 1024], F32, tag="y2", name="y2")
                        for o, w in ((0, 512), (512, 256)):
                            for j in range(FP8_PAIRS):
                                nc.tensor.matmul(
                                    y2p[:, o : o + w],
                                    g8[:, 2 * j : 2 * j + 2, iq * 128 : (iq + 1) * 128],
                                    w28sb[:, j, :, o : o + w],
                                    start=(j == 0), stop=(j == FP8_PAIRS - 1 and NB16 == 0),
                                    perf_mode=DR, skip_group_check=True,
                                )
                            for cb in range(NB16):
                                nc.tensor.matmul(
                                    y2p[:, o : o + w],
                                    gb[:, cb, iq * 128 : (iq + 1) * 128],
                                    w2sb[:, cb, o : o + w],
                                    start=(FP8_PAIRS == 0 and cb == 0),
                                    stop=(cb == NB16 - 1),
                                    skip_group_check=True,
                                )
                        t2 = ln2_pool.tile([128, E], F32, tag="t2", name="t2")
                        x1q = x1b[:, qt, :]
                        if not zero_bias:
                            x1f = ln2_pool.tile([128, E], F32, tag="x1f", name="x1f")
                            nc.vector.tensor_tensor(out=x1f[:], in0=x1q, in1=bce[:, 0, :], op=ALU.add)
                            x1q = x1f[:]
                        nc.vector.scalar_tensor_tensor(
                            out=t2[:], in0=y2p[:, 0:E], scalar=FF2_DEQ, in1=x1q,
                            op0=ALU.mult, op1=ALU.add,
                        )
                        yout = out_pool.tile([128, E], F32, tag="yout", name="yout")
                        _ln_stats_norm(nc, ln2_pool, t2[:], yout[:], eps_t,
                                       bce[:, 1, :] if bce is not None else None,
                                       bce[:, 2, :] if bce is not None else None,
                                       identity_ln, "ln2")
                        nc.sync.dma_start(out=y_d[qt * 128 : (qt + 1) * 128, :], in_=yout[:])

    nc.compile()
    return nc


_NC_CACHE = {}


def _get_nc(identity_ln=False, zero_bias=False):
    key = (identity_ln, zero_bias)
    if key not in _NC_CACHE:
        _NC_CACHE[key] = _build(identity_ln, zero_bias)
    return _NC_CACHE[key]


def _q8(a):
    return np.asarray(a, dtype=ml_dtypes.float8_e4m3fn)


def _prep_weights(w_qkvr, b_qkvr, w_proj, b_proj, ln1_g, ln1_b,
                  w_ff1, b_ff1, w_ff2, b_ff2, ln2_g, ln2_b):
    w4 = np.asarray(w_qkvr, np.float32).reshape(E, H, D, 4)
    b4 = np.asarray(b_qkvr, np.float32).reshape(H, D, 4)

    wqkr = np.zeros((128, H, 3, 3, 2, D), np.float32)
    for ti, t in enumerate((0, 1, 3)):
        wt = w4[..., t] * WS
        for kc in range(3):
            for i in range(2):
                wqkr[:, :, ti, kc, i, :] = wt[256 * kc + 128 * i : 256 * kc + 128 * (i + 1)]
    wvf = w4[..., 2].reshape(E, E) * WS
    wv = np.zeros((128, 3, 2, E), np.float32)
    for kc in range(3):
        for i in range(2):
            wv[:, kc, i, :] = wvf[256 * kc + 128 * i : 256 * kc + 128 * (i + 1)]
    wpf = np.asarray(w_proj, np.float32) * WS
    wp = np.zeros((D, 4, 2, E), np.float32)
    for hp in range(4):
        for i in range(2):
            wp[:, hp, i, :] = wpf[(2 * hp + i) * D : (2 * hp + i + 1) * D]
    bqkr = np.stack([b4[..., 0], b4[..., 1], b4[..., 3] * WS], 0).transpose(2, 0, 1)
    bv = np.ascontiguousarray(
        (b4[..., 2] * (VA_SCALE / np.sqrt(E))).reshape(1, E)).astype(np.float32)
    w1f = np.asarray(w_ff1, np.float32) * WS
    wff1 = np.zeros((128, NCT, 3, 2, 128), np.float32)
    w1r = w1f.reshape(3, 2, 128, NCT, 128)
    wff1[:] = w1r.transpose(2, 3, 0, 1, 4)
    bff1 = np.ascontiguousarray(np.asarray(b_ff1, np.float32).reshape(NCT, 128).T)
    w2s = np.asarray(w_ff2, np.float32) * FF2_SCALE
    # wff28: [p, j, i, n] = 512*w2[256j+128i+p, n]
    w28 = np.ascontiguousarray(
        w2s.reshape(NCT // 2, 2, 128, E).transpose(2, 0, 1, 3))
    return {
        "wqkr": _q8(wqkr), "wv": _q8(wv), "wproj": _q8(wp),
        "bqkr": np.ascontiguousarray(bqkr), "bv": bv,
        "bproj": np.asarray(b_proj, np.float32).reshape(1, E).copy(),
        "ln1g": np.asarray(ln1_g, np.float32).reshape(1, E).copy(),
        "ln1b": np.asarray(ln1_b, np.float32).reshape(1, E).copy(),
        "wff1": _q8(wff1), "bff1": bff1,
        "wff2": np.ascontiguousarray(w2s).astype(ml_dtypes.bfloat16),
        "wff28": _q8(w28),
        "bff2": np.asarray(b_ff2, np.float32).reshape(1, E).copy(),
        "ln2g": np.asarray(ln2_g, np.float32).reshape(1, E).copy(),
        "ln2b": np.asarray(ln2_b, np.float32).reshape(1, E).copy(),
    }


def _in_maps(inputs):
    x = np.asarray(inputs["x"], np.float32)
    shared = _prep_weights(
        inputs["w_qkvr"], inputs["b_qkvr"], inputs["w_proj"], inputs["b_proj"],
        inputs["ln1_g"], inputs["ln1_b"], inputs["w_ff1"], inputs["b_ff1"],
        inputs["w_ff2"], inputs["b_ff2"], inputs["ln2_g"], inputs["ln2_b"],
    )
    maps = []
    for i in range(N_CORES):
        xi = x[i]
        x8 = np.asarray(xi, ml_dtypes.float8_e4m3fn)
        x8t = np.ascontiguousarray(x8.T.reshape(NEC, 128, N).transpose(1, 0, 2))
        xb16 = np.ascontiguousarray(xi.astype(ml_dtypes.bfloat16))
        maps.append({**shared, "x8t": x8t, "xb16": xb16})
    return maps


def _flags(inputs):
    z = lambda k: not np.any(np.asarray(inputs[k]))
    one = lambda k: bool(np.all(np.asarray(inputs[k]) == 1.0))
    identity_ln = (one("ln1_g") and z("ln1_b") and one("ln2_g") and z("ln2_b"))
    zero_bias = (z("b_qkvr") and z("b_proj") and z("b_ff2"))
    return identity_ln, zero_bias


def kernel(**inputs) -> np.ndarray:
    identity_ln, zero_bias = _flags(inputs)
    nc = _get_nc(identity_ln, zero_bias)
    res = run_bass_kernel_spmd(nc, _in_maps(inputs), core_ids=list(range(N_CORES)))
    return np.stack([res.results[i]["y"] for i in range(N_CORES)], axis=0)


# revision 15
# speedup vs baseline: 1.0212x; 1.0212x over previous
"""Trainium2 Bass kernel v4 for the encoder block.

Data-parallel over batch (1 element/core, no collectives).

vs v3:
- DMAs coalesced (one trigger per tensor; the sync sequencer costs
  ~650ns per dma_start) and ordered so x8T/wv8 land first; bulky
  ff-weights stream during attention.
- v_aug pad memsets moved to gpsimd and shrunk to the pad columns.
- proj/LN1 (phase D) interleaved with ff1 halves to keep the tensor
  engine fed across the transition; LN applies run on gpsimd and
  x1T8 psum evacuations on ACT to unload DVE.
- ff2 contraction split: first 2*FP8_PAIRS ct-chunks run fp8
  DoubleRow (gelu emits unscaled fp8 g; w2 pre-scaled by 512), the
  remaining chunks stay bf16 (also 512x so the psum scale matches).
  FP8_PAIRS=9 keeps rel-err ~1.7e-2 (sim) while cutting ff2 ~37%.

Scale bookkeeping: x8T = fp8(x)^T; wq/k/r/v/proj/ff1 = fp8(32*w);
q,k evac 1/32 (bf16 true), r keeps 32x; v_aug8 = fp8(2048*v/sqrt(E)),
ones col 32; og8 = psum_d*r_sb/(32*psum_one) = 64*(r*og)_true;
proj dequant 1/2048 fused into residual; ff2 psum = 512*ff ->
dequant 1/512 fused into residual.
"""

import sys

if "/opt/trn_rl_repo" not in sys.path:
    sys.path.insert(0, "/opt/trn_rl_repo")

from contextlib import ExitStack

import numpy as np
import ml_dtypes

import concourse.bass as bass
import concourse.mybir as mybir
import concourse.tile as tile
from concourse import bacc
from concourse.bass_utils import run_bass_kernel_spmd
from concourse.masks import make_identity

F32 = mybir.dt.float32
BF16 = mybir.dt.bfloat16
FP8 = mybir.dt.float8e4
AF = mybir.ActivationFunctionType
ALU = mybir.AluOpType
DR = mybir.MatmulPerfMode.DoubleRow

N_CORES = 8
B, N, E = 8, 1024, 768
H, D = 8, 96
C = 4 * E
NQT = N // 128
NEC = E // 128
NCT = C // 128
LN_EPS = 1e-5

WS = 32.0
INV_WS = 1.0 / WS
C_SHIFT = 12.0       # exp(e - C_SHIFT): emax ~17.3 -> exp' <= ~210 < 448
ONES_VAL = 32.0
VA_SCALE = 2048.0
SUM_EPS = 1e-6
PROJ_DEQ = 1.0 / (64.0 * 32.0)
FF2_SCALE = 512.0    # w2 pre-scale (both fp8 and bf16 parts)
FF2_DEQ = 1.0 / FF2_SCALE
FP8_PAIRS = 9        # ct-pairs of ff2 contraction done in fp8 DR (0..12)


def _bcast_dma(nc, out_ap, row_ap):
    src = bass.AP(
        tensor=row_ap.tensor,
        offset=row_ap.offset,
        ap=[[0, out_ap.shape[0]], list(row_ap.ap[-1])],
    )
    nc.gpsimd.dma_start(out=out_ap, in_=src)


def _ln_stats_norm(nc, pool, t1, out, eps_t, g_bc, b_bc, identity_ln, tag):
    """LN over free dim 768 of t1 -> out. Stats on DVE, apply on gpsimd."""
    scr = pool.tile([128, 32], F32, tag=f"lns_{tag}", name=f"lns_{tag}")
    st = scr[:, 0:18].rearrange("p (a b) -> p a b", a=3)
    mv = scr[:, 24:26]
    rstd = scr[:, 26:27]
    t2 = out if identity_ln else pool.tile([128, E], F32, tag=f"lnt2_{tag}", name=f"lnt2_{tag}")
    for sg in range(3):
        nc.vector.bn_stats(st[:, sg, :], t1[:, sg * 256 : (sg + 1) * 256])
    nc.vector.bn_aggr(mv, st)
    nc.scalar.activation(out=rstd, in_=mv[:, 1:2], func=AF.Sqrt, bias=eps_t[:], scale=1.0)
    nc.vector.reciprocal(rstd, rstd)
    nc.vector.tensor_scalar(
        out=t2[:] if t2 is not out else t2, in0=t1, scalar1=mv[:, 0:1], scalar2=rstd,
        op0=ALU.subtract, op1=ALU.mult,
    )
    if not identity_ln:
        nc.vector.tensor_tensor(out=t2[:], in0=t2[:], in1=g_bc, op=ALU.mult)
        nc.vector.tensor_tensor(out=out, in0=t2[:], in1=b_bc, op=ALU.add)


def _build(identity_ln=False, zero_bias=False):
    nc = bacc.Bacc(num_devices=N_CORES)
    NF8 = 2 * FP8_PAIRS          # ct chunks done in fp8
    NB16 = NCT - NF8             # ct chunks done in bf16

    x8t_d = nc.declare_dram_parameter("x8t", [128, NEC, N], FP8, isOutput=False)
    xb16_d = nc.declare_dram_parameter("xb16", [N, E], BF16, isOutput=False)
    wqkr_d = nc.declare_dram_parameter("wqkr", [128, H, 3, 3, 2, D], FP8, isOutput=False)
    wv_d = nc.declare_dram_parameter("wv", [128, 3, 2, E], FP8, isOutput=False)
    wproj_d = nc.declare_dram_parameter("wproj", [D, 4, 2, E], FP8, isOutput=False)
    bqkr_d = nc.declare_dram_parameter("bqkr", [D, 3, H], F32, isOutput=False)
    bv_d = nc.declare_dram_parameter("bv", [1, E], F32, isOutput=False)
    bproj_d = nc.declare_dram_parameter("bproj", [1, E], F32, isOutput=False)
    ln1g_d = nc.declare_dram_parameter("ln1g", [1, E], F32, isOutput=False)
    ln1b_d = nc.declare_dram_parameter("ln1b", [1, E], F32, isOutput=False)
    wff1_d = nc.declare_dram_parameter("wff1", [128, NCT, 3, 2, 128], FP8, isOutput=False)
    bff1_d = nc.declare_dram_parameter("bff1", [128, NCT], F32, isOutput=False)
    wff2_d = nc.declare_dram_parameter("wff2", [C, E], BF16, isOutput=False)
    wff28_d = nc.declare_dram_parameter("wff28", [128, NCT // 2, 2, E], FP8, isOutput=False)
    bff2_d = nc.declare_dram_parameter("bff2", [1, E], F32, isOutput=False)
    ln2g_d = nc.declare_dram_parameter("ln2g", [1, E], F32, isOutput=False)
    ln2b_d = nc.declare_dram_parameter("ln2b", [1, E], F32, isOutput=False)
    y_d = nc.declare_dram_parameter("y", [N, E], F32, isOutput=True)

    with tile.TileContext(nc) as tc, ExitStack() as ctx:
        persist = ctx.enter_context(tc.tile_pool(name="persist", bufs=1))
        vaug_pool = ctx.enter_context(tc.tile_pool(name="vaug", bufs=1))
        og_pool = ctx.enter_context(tc.tile_pool(name="og", bufs=1))
        x1_pool = ctx.enter_context(tc.tile_pool(name="x1", bufs=1))
        x1t_pool = ctx.enter_context(tc.tile_pool(name="x1t", bufs=1))

        # critical-path DMAs first (one trigger each)
        x8T = persist.tile([128, NEC, N], FP8, name="x8T")
        nc.sync.dma_start(out=x8T[:], in_=x8t_d[:])
        wv8 = persist.tile([128, 3, 2, E], FP8, name="wv8")
        nc.sync.dma_start(out=wv8[:], in_=wv_d[:])
        bqkr_t = persist.tile([D, 3, H], F32)
        nc.sync.dma_start(out=bqkr_t[:], in_=bqkr_d[:])
        wp8 = persist.tile([D, 4, 2, E], FP8, name="wp8")
        nc.sync.dma_start(out=wp8[:], in_=wproj_d[:])

        identb = persist.tile([128, 128], BF16)
        make_identity(nc, identb[:])
        eps_t = persist.tile([128, 1], F32)
        nc.vector.memset(eps_t[:], LN_EPS)
        shift_t = persist.tile([128, 1], F32)
        nc.vector.memset(shift_t[:], -C_SHIFT)

        # PE warm-up while DMAs land
        warm_t = persist.tile([128, 128], BF16)
        nc.vector.memset(warm_t[:], 0.0)
        with tc.tile_pool(name="warm_ps", bufs=2, space="PSUM") as warm_ps:
            for _ in range(8):
                wp_ = warm_ps.tile([128, 128], F32, tag="wp_", name="wp_")
                nc.tensor.matmul(wp_[:], warm_t[:], warm_t[:], start=True, stop=True)
                nc.tensor.matmul(wp_[:], warm_t[:], warm_t[:], start=True, stop=True)

        # v_aug pad + ones cols (gpsimd, off the DVE critical path)
        v_aug = vaug_pool.tile([128, H, NQT, 128], FP8, name="v_aug")
        nc.gpsimd.memset(v_aug[:, :, :, D:], 0.0)
        nc.gpsimd.memset(v_aug[:, :, :, D : D + 1], ONES_VAL)

        # bulky ff-phase loads: issue now, they stream during attention
        xb16 = persist.tile([128, NQT, E], BF16, name="xb16")
        src = bass.AP(tensor=xb16_d.tensor if hasattr(xb16_d, 'tensor') else xb16_d, offset=0,
                      ap=[[E, 128], [128 * E, NQT], [1, E]])
        nc.sync.dma_start(out=xb16[:], in_=src)
        wff1sb = persist.tile([128, NCT, 3, 2, 128], FP8, name="wff1sb")
        nc.sync.dma_start(out=wff1sb[:], in_=wff1_d[:])
        bff1_t = persist.tile([128, NCT], F32, name="bff1_t")
        nc.sync.dma_start(out=bff1_t[:], in_=bff1_d[:])
        w28sb = None
        if FP8_PAIRS > 0:
            w28sb = persist.tile([128, FP8_PAIRS, 2, E], FP8, name="w28sb")
            nc.sync.dma_start(out=w28sb[:], in_=wff28_d[:, 0:FP8_PAIRS])
        NF8_ = NF8
        w2sb = None
        if NB16 > 0:
            w2sb = persist.tile([128, NB16, E], BF16, name="w2sb")
            src2 = bass.AP(tensor=wff2_d.tensor if hasattr(wff2_d, 'tensor') else wff2_d,
                           offset=NF8_ * 128 * E,
                           ap=[[E, 128], [128 * E, NB16], [1, E]])
            nc.sync.dma_start(out=w2sb[:], in_=src2)

        # ---- phase B: v = x @ wv (fp8 DR) -> v_aug8 ----
        VSC = VA_SCALE / (WS * float(np.sqrt(E)))
        with (
            tc.tile_pool(name="bcv", bufs=1) as bcv_pool,
            tc.tile_pool(name="v_ps", bufs=2, space="PSUM") as v_ps,
        ):
            bv_bc = None
            if not zero_bias:
                bv_bc = bcv_pool.tile([128, E], F32, tag="bv", name="bv_bc")
                _bcast_dma(nc, bv_bc[:], bv_d[0:1, :])
            for qt in range(NQT):
                vps = v_ps.tile([128, 1024], F32, tag="vp", name="vp")
                for o, w in ((0, 512), (512, 256)):
                    for kc in range(3):
                        nc.tensor.matmul(
                            vps[:, o : o + w],
                            x8T[:, 2 * kc : 2 * kc + 2, qt * 128 : (qt + 1) * 128],
                            wv8[:, kc, :, o : o + w],
                            start=(kc == 0), stop=(kc == 2), perf_mode=DR,
                        )
                dst = v_aug[:, :, qt, 0:D]
                src_v = vps[:, 0:E].rearrange("p (h d) -> p h d", h=H)
                if zero_bias:
                    nc.vector.tensor_scalar(
                        out=dst, in0=src_v, scalar1=VSC, scalar2=None, op0=ALU.mult,
                    )
                else:
                    nc.vector.scalar_tensor_tensor(
                        out=dst, in0=src_v, scalar=VSC,
                        in1=bv_bc[:, 0:E].rearrange("p (h d) -> p h d", h=H),
                        op0=ALU.mult, op1=ALU.add,
                    )

        # ---- phase C: attention, software-pipelined over heads ----
        og8 = og_pool.tile([D, H, N], FP8, name="og8")
        with (
            tc.tile_pool(name="wqkr", bufs=2) as wqkr_pool,
            tc.tile_pool(name="qkr", bufs=2) as qkr_pool,
            tc.tile_pool(name="expE", bufs=2) as exp_pool,
            tc.tile_pool(name="att_tmp", bufs=2) as tmp_pool,
            tc.tile_pool(name="qkr_ps", bufs=2, space="PSUM") as qkr_ps,
            tc.tile_pool(name="eng_ps", bufs=2, space="PSUM") as eng_ps,
            tc.tile_pool(name="att_ps", bufs=1, space="PSUM") as att_ps,
        ):
            def qkr_stage(h):
                w_sb = wqkr_pool.tile([128, 3, 3, 2, D], FP8, tag="w_qkr", name="w_qkr")
                nc.sync.dma_start(out=w_sb[:], in_=wqkr_d[:, h])
                qkrT = {}
                for si, name in enumerate(("q", "k", "r")):
                    dst = qkr_pool.tile([D, N], BF16, tag=f"{name}T", name=f"{name}T")
                    qkrT[name] = dst
                    for half in range(2):
                        ps = qkr_ps.tile([D, 512], F32, tag="qkrp", name="qkrp")
                        for kc in range(3):
                            nc.tensor.matmul(
                                ps[:],
                                w_sb[:, si, kc],
                                x8T[:, 2 * kc : 2 * kc + 2,
                                    half * 512 : (half + 1) * 512],
                                start=(kc == 0), stop=(kc == 2), perf_mode=DR,
                            )
                        out_sl = dst[:, half * 512 : (half + 1) * 512]
                        if name == "r":
                            if zero_bias:
                                nc.vector.tensor_copy(out_sl, ps[:])
                            else:
                                nc.vector.tensor_scalar(
                                    out=out_sl, in0=ps[:],
                                    scalar1=bqkr_t[:, si, h : h + 1], scalar2=None,
                                    op0=ALU.add,
                                )
                        else:
                            if zero_bias:
                                nc.vector.tensor_scalar(
                                    out=out_sl, in0=ps[:],
                                    scalar1=INV_WS, scalar2=None, op0=ALU.mult,
                                )
                            else:
                                nc.vector.tensor_scalar(
                                    out=out_sl, in0=ps[:],
                                    scalar1=INV_WS, scalar2=bqkr_t[:, si, h : h + 1],
                                    op0=ALU.mult, op1=ALU.add,
                                )
                return qkrT

            def energy_stage(h, qkrT):
                expE = exp_pool.tile([128, NQT, N], FP8, tag="expE", name="expE")
                for kt in range(NQT):
                    ep = eng_ps.tile([128, N], F32, tag="ep", name="ep")
                    for qh in range(2):
                        nc.tensor.matmul(
                            ep[:, qh * 512 : (qh + 1) * 512],
                            qkrT["k"][:, kt * 128 : (kt + 1) * 128],
                            qkrT["q"][:, qh * 512 : (qh + 1) * 512],
                            start=True, stop=True, skip_group_check=True,
                        )
                    nc.scalar.activation(
                        out=expE[:, kt, :], in_=ep[:], func=AF.Exp,
                        bias=shift_t[:], scale=1.0,
                    )
                return expE

            def attv_stage(h, expE):
                op_ = att_ps.tile([128, N], F32, tag="op", name="op")
                for j in range(NQT // 2):
                    for qh in range(2):
                        nc.tensor.matmul(
                            op_[:, qh * 512 : (qh + 1) * 512],
                            v_aug[:, h, 2 * j : 2 * j + 2, :],
                            expE[:, 2 * j : 2 * j + 2, qh * 512 : (qh + 1) * 512],
                            start=(j == 0), stop=(j == NQT // 2 - 1),
                            perf_mode=DR, skip_group_check=True,
                        )
                return op_

            def tail_stage(h, op_, qkrT):
                su = tmp_pool.tile([1, N], F32, tag="su", name="su")
                nc.vector.tensor_scalar(
                    out=su[:], in0=op_[D : D + 1, :],
                    scalar1=32.0, scalar2=SUM_EPS, op0=ALU.mult, op1=ALU.add,
                )
                gated = tmp_pool.tile([D, N], F32, tag="gated", name="gated")
                nc.vector.tensor_tensor(
                    out=gated[:], in0=op_[0:D, :], in1=qkrT["r"][:], op=ALU.mult,
                )
                rcp = tmp_pool.tile([1, N], F32, tag="rcp", name="rcp")
                nc.vector.reciprocal_approx_fast(rcp[:], su[:])
                bcr = tmp_pool.tile([D, N], F32, tag="bcr", name="bcr")
                nc.gpsimd.partition_broadcast(bcr[:], rcp[:])
                nc.vector.tensor_tensor(
                    out=og8[:, h, :], in0=gated[:], in1=bcr[:], op=ALU.mult,
                )

            prev = None
            for h in range(H):
                qkrT = qkr_stage(h)
                if prev is not None:
                    ph, pexpE, pqkrT = prev
                    op_ = attv_stage(ph, pexpE)
                    tail_stage(ph, op_, pqkrT)
                expE = energy_stage(h, qkrT)
                prev = (h, expE, qkrT)
            ph, pexpE, pqkrT = prev
            op_ = attv_stage(ph, pexpE)
            tail_stage(ph, op_, pqkrT)

        # ---- phases D+E interleaved: proj+LN1 / ff1 / ff2+LN2 ----
        x1b = x1_pool.tile([128, NQT, E], BF16, name="x1b")
        x1T8 = x1t_pool.tile([128, NEC, N], FP8, name="x1T8")
        with (
            tc.tile_pool(name="bcmid", bufs=1) as bcm_pool,
            tc.tile_pool(name="ln_tmp", bufs=2) as ln_pool,
            tc.tile_pool(name="gs8", bufs=2) as gs8_pool,
            tc.tile_pool(name="gsb", bufs=2) as gsb_pool,
            tc.tile_pool(name="ln2_tmp", bufs=2) as ln2_pool,
            tc.tile_pool(name="out", bufs=2) as out_pool,
            tc.tile_pool(name="h1_ps", bufs=3, space="PSUM") as h1_ps,
        ):
            bcm = bce = None
            if not (identity_ln and zero_bias):
                bcm = bcm_pool.tile([128, 3, E], F32, name="bcm")
                for i, d in enumerate((bproj_d, ln1g_d, ln1b_d)):
                    _bcast_dma(nc, bcm[:, i, :], d[0:1, :])
                bce = bcm_pool.tile([128, 3, E], F32, name="bce")
                for i, d in enumerate((bff2_d, ln2g_d, ln2b_d)):
                    _bcast_dma(nc, bce[:, i, :], d[0:1, :])

            def proj_ln(qt, y1_ps, tp1_ps):
                yp = y1_ps.tile([128, 1024], F32, tag="yp", name="yp")
                for o, w in ((0, 512), (512, 256)):
                    for hp in range(4):
                        nc.tensor.matmul(
                            yp[:, o : o + w],
                            og8[:, 2 * hp : 2 * hp + 2, qt * 128 : (qt + 1) * 128],
                            wp8[:, hp, :, o : o + w],
                            start=(hp == 0), stop=(hp == 3), perf_mode=DR,
                        )
                t1 = ln_pool.tile([128, E], F32, tag="t1", name="t1")
                xr = xb16[:, qt, :]
                if not zero_bias:
                    xrf = ln_pool.tile([128, E], F32, tag="xrf", name="xrf")
                    nc.vector.tensor_tensor(out=xrf[:], in0=xr, in1=bcm[:, 0, :], op=ALU.add)
                    xr = xrf[:]
                nc.vector.scalar_tensor_tensor(
                    out=t1[:], in0=yp[:, 0:E], scalar=PROJ_DEQ, in1=xr,
                    op0=ALU.mult, op1=ALU.add,
                )
                _ln_stats_norm(nc, ln_pool, t1[:], x1b[:, qt, :], eps_t,
                               bcm[:, 1, :] if bcm is not None else None,
                               bcm[:, 2, :] if bcm is not None else None,
                               identity_ln, "ln1")
                pt1 = tp1_ps.tile([128, NEC, 128], BF16, tag="pt1", name="pt1")
                for ec in range(NEC):
                    nc.tensor.transpose(pt1[:, ec, :], x1b[:, qt, ec * 128 : (ec + 1) * 128], identb[:])
                nc.scalar.activation(
                    out=x1T8[:, :, qt * 128 : (qt + 1) * 128], in_=pt1[:], func=AF.Copy,
                )

            def ff1_half(half):
                g8 = gs8_pool.tile([128, NF8, 512], FP8, tag="g8", name="g8") if NF8 else None
                gb = gsb_pool.tile([128, NB16, 512], BF16, tag="gb", name="gb") if NB16 else None
                for ct in range(NCT):
                    hp_ = h1_ps.tile([128, 512], F32, tag="h1", name="h1")
                    for kc in range(3):
                        nc.tensor.matmul(
                            hp_[:],
                            wff1sb[:, ct, kc],
                            x1T8[:, 2 * kc : 2 * kc + 2,
                                 half * 512 : (half + 1) * 512],
                            start=(kc == 0), stop=(kc == 2), perf_mode=DR,
                        )
                    if ct < NF8:
                        gdst = g8[:, ct, :]
                    else:
                        gdst = gb[:, ct - NF8, :]
                    nc.scalar.activation(
                        out=gdst, in_=hp_[:], func=AF.Gelu,
                        bias=bff1_t[:, ct : ct + 1], scale=INV_WS,
                    )
                return g8, gb

            gparts = [None, None]
            with (
                tc.tile_pool(name="y1_ps", bufs=2, space="PSUM") as y1_ps,
                tc.tile_pool(name="tp1_ps", bufs=1, space="PSUM") as tp1_ps,
            ):
                for qt in range(NQT):
                    proj_ln(qt, y1_ps, tp1_ps)
            gparts[0] = ff1_half(0)
            gparts[1] = ff1_half(1)

            with tc.tile_pool(name="y2_ps", bufs=2, space="PSUM") as y2_ps:
                for half in range(2):
                    g8, gb = gparts[half]
                    for iq in range(4):
                        qt = half * 4 + iq
                        y2p = y2_ps.tile([128, 1024], F32, tag="y2", name="y2")
                        for o, w in ((0, 512), (512, 256)):
                            for j in range(FP8_PAIRS):
                                nc.tensor.matmul(
                                    y2p[:, o : o + w],
                                    g8[:, 2 * j : 2 * j + 2, iq * 128 : (iq + 1) * 128],
                                    w28sb[:, j, :, o : o + w],
                                    start=(j == 0), stop=(j == FP8_PAIRS - 1 and NB16 == 0),
                                    perf_mode=DR, skip_group_check=True,
                                )
                            for cb in range(NB16):
                                nc.tensor.matmul(
                                    y2p[:, o : o + w],
                                    gb[:, cb, iq * 128 : (iq + 1) * 128],
                                    w2sb[:, cb, o : o + w],
                                    start=(FP8_PAIRS == 0 and cb == 0),
                                    stop=(cb == NB16 - 1),
                                    skip_group_check=True,
                                )
                        t2 = ln2_pool.tile([128, E], F32, tag="t2", name="t2")
                        x1q = x1b[:, qt, :]
                        if not zero_bias:
                            x1f = ln2_pool.tile([128, E], F32, tag="x1f", name="x1f")
                            nc.vector.tensor_tensor(out=x1f[:], in0=x1q, in1=bce[:, 0, :], op=ALU.add)
                            x1q = x1f[:]
                        nc.vector.scalar_tensor_tensor(
                            out=t2[:], in0=y2p[:, 0:E], scalar=FF2_DEQ, in1=x1q,
                            op0=ALU.mult, op1=ALU.add,
                        )
                        yout = out_pool.tile([128, E], F32, tag="yout", name="yout")
                        _ln_stats_norm(nc, ln2_pool, t2[:], yout[:], eps_t,
                                       bce[:, 1, :] if bce is not None else None,
                                       bce[:, 2, :] if bce is not None else None,
                                       identity_ln, "ln2")
                        nc.sync.dma_start(out=y_d[qt * 128 : (qt + 1) * 128, :], in_=yout[:])

    nc.compile()
    return nc


_NC_CACHE = {}


def _get_nc(identity_ln=False, zero_bias=False):
    key = (identity_ln, zero_bias)
    if key not in _NC_CACHE:
        _NC_CACHE[key] = _build(identity_ln, zero_bias)
    return _NC_CACHE[key]


def _q8(a):
    return np.asarray(a, dtype=ml_dtypes.float8_e4m3fn)


def _prep_weights(w_qkvr, b_qkvr, w_proj, b_proj, ln1_g, ln1_b,
                  w_ff1, b_ff1, w_ff2, b_ff2, ln2_g, ln2_b):
    w4 = np.asarray(w_qkvr, np.float32).reshape(E, H, D, 4)
    b4 = np.asarray(b_qkvr, np.float32).reshape(H, D, 4)

    wqkr = np.zeros((128, H, 3, 3, 2, D), np.float32)
    for ti, t in enumerate((0, 1, 3)):
        wt = w4[..., t] * WS
        for kc in range(3):
            for i in range(2):
                wqkr[:, :, ti, kc, i, :] = wt[256 * kc + 128 * i : 256 * kc + 128 * (i + 1)]
    wvf = w4[..., 2].reshape(E, E) * WS
    wv = np.zeros((128, 3, 2, E), np.float32)
    for kc in range(3):
        for i in range(2):
            wv[:, kc, i, :] = wvf[256 * kc + 128 * i : 256 * kc + 128 * (i + 1)]
    wpf = np.asarray(w_proj, np.float32) * WS
    wp = np.zeros((D, 4, 2, E), np.float32)
    for hp in range(4):
        for i in range(2):
            wp[:, hp, i, :] = wpf[(2 * hp + i) * D : (2 * hp + i + 1) * D]
    bqkr = np.stack([b4[..., 0], b4[..., 1], b4[..., 3] * WS], 0).transpose(2, 0, 1)
    bv = np.ascontiguousarray(
        (b4[..., 2] * (VA_SCALE / np.sqrt(E))).reshape(1, E)).astype(np.float32)
    w1f = np.asarray(w_ff1, np.float32) * WS
    wff1 = np.zeros((128, NCT, 3, 2, 128), np.float32)
    w1r = w1f.reshape(3, 2, 128, NCT, 128)
    wff1[:] = w1r.transpose(2, 3, 0, 1, 4)
    bff1 = np.ascontiguousarray(np.asarray(b_ff1, np.float32).reshape(NCT, 128).T)
    w2s = np.asarray(w_ff2, np.float32) * FF2_SCALE
    # wff28: [p, j, i, n] = 512*w2[256j+128i+p, n]
    w28 = np.ascontiguousarray(
        w2s.reshape(NCT // 2, 2, 128, E).transpose(2, 0, 1, 3))
    return {
        "wqkr": _q8(wqkr), "wv": _q8(wv), "wproj": _q8(wp),
        "bqkr": np.ascontiguousarray(bqkr), "bv": bv,
        "bproj": np.asarray(b_proj, np.float32).reshape(1, E).copy(),
        "ln1g": np.asarray(ln1_g, np.float32).reshape(1, E).copy(),
        "ln1b": np.asarray(ln1_b, np.float32).reshape(1, E).copy(),
        "wff1": _q8(wff1), "bff1": bff1,
        "wff2": np.ascontiguousarray(w2s).astype(ml_dtypes.bfloat16),
        "wff28": _q8(w28),
        "bff2": np.asarray(b_ff2, np.float32).reshape(1, E).copy(),
        "ln2g": np.asarray(ln2_g, np.float32).reshape(1, E).copy(),
        "ln2b": np.asarray(ln2_b, np.float32).reshape(1, E).copy(),
    }


def _in_maps(inputs):
    x = np.asarray(inputs["x"], np.float32)
    shared = _prep_weights(
        inputs["w_qkvr"], inputs["b_qkvr"], inputs["w_proj"], inputs["b_proj"],
        inputs["ln1_g"], inputs["ln1_b"], inputs["w_ff1"], inputs["b_ff1"],
        inputs["w_ff2"], inputs["b_ff2"], inputs["ln2_g"], inputs["ln2_b"],
    )
    maps = []
    for i in range(N_CORES):
        xi = x[i]
        x8 = np.asarray(xi, ml_dtypes.float8_e4m3fn)
        x8t = np.ascontiguousarray(x8.T.reshape(NEC, 128, N).transpose(1, 0, 2))
        xb16 = np.ascontiguousarray(xi.astype(ml_dtypes.bfloat16))
        maps.append({**shared, "x8t": x8t, "xb16": xb16})
    return maps


def _flags(inputs):
    z = lambda k: not np.any(np.asarray(inputs[k]))
    one = lambda k: bool(np.all(np.asarray(inputs[k]) == 1.0))
    identity_ln = (one("ln1_g") and z("ln1_b") and one("ln2_g") and z("ln2_b"))
    zero_bias = (z("b_qkvr") and z("b_proj") and z("b_ff2"))
    return identity_ln, zero_bias


def kernel(**inputs) -> np.ndarray:
    identity_ln, zero_bias = _flags(inputs)
    nc = _get_nc(identity_ln, zero_bias)
    res = run_bass_kernel_spmd(nc, _in_maps(inputs), core_ids=list(range(N_CORES)))
    return np.stack([res.results[i]["y"] for i in range(N_CORES)], axis=0)
